# revision 20
# baseline (speedup 1.0000x reference)
"""Multi-head attention (B=2, S=2048, D=1024, H=16) on 8 TRN2 NeuronCores.

Sharding: data-parallel over batch (2 groups of 4 cores), tensor-parallel over
heads within a group (4 heads = 256 feature columns per core). Each core:
  - projects its batch's q/k/v (full D contraction) into its 256-col head slice
  - runs full attention for its 4 heads over the 2048-token sequence
  - applies its 256-row slice of w_o, producing a partial [D, S] output (bf16)
Host sums the 4 partials per batch (+ b_o folded into one core per batch) and
transposes back to [S, D].

Structure (226us baseline -> this version):
  - flat conveyor of per-tt "turns" (scores -> exp -> P@V) across all 8
    (sb, jt) pairs, grouped TWO turns at a time: [SC SC][weave][PV PV PV PV].
    Scores pairs adjacent halves amortize the row-tiled weight-load
    serialization; the full-width weave matmul after them eats the
    transition cost while doing useful work.
  - v-projection emits P@V-ready [token, feature] tiles directly
    (lhsT=x chunk, rhs=w_v tile) -- no PE transposes, no vector repack
    copies. b_v is folded into b_o on the host (b_o_eff = b_o + W_o^T b_v,
    exact), so the v evac is a single strided copy.
  - endgame: the last pair's softmax norm reads PSUM directly (no usb copy,
    no rc DMA), head3 lands in a base-0 tmp tile and the final out-proj
    contracts it with a separately-loaded w_o row-slice (2x K=64 matmuls),
    avoiding the 3us SBUF->SBUF partition-offset DMA; final output DMA is
    split per-ft across 8 queues; a few identity transposes keep the PE
    clock up across the norm latency.
  - mid-kernel psU release copies run on GpSimd (idle) instead of Vector.
Softmax denominator comes from a ones-column appended to each head's V tile
(PSUM-accumulated by the P@V matmul).
"""

import numpy as np

B, S, D, H = 2, 2048, 1024, 16
DK = D // H          # 64
NCORES = 8
GROUPS = 4           # head-groups (cores) per batch
JC = D // GROUPS     # 256 feature columns per core (4 heads)
TB = 512             # token block (matmul moving free dim)
NTB = S // TB        # 4
NDT = D // 128       # 8 contraction tiles for projections
NTT = S // 128       # 16 key-token tiles per sequence
VROW = 2 * (DK + 1)  # 130: per-jt vp row segment (2 heads x (64 v cols + ones))

# Warmup transposes do NOT ramp the PE clock (the HAM ramp tracks ~3.4us of
# sustained real matmul work); they only prevent the idle gate-down (~2us of
# idle drops the clock back to 1.2 GHz). So: almost none at startup (the head
# is DMA-bound at low clock regardless), many across the endgame norm-chain
# latency where the clock is up and must stay up.
W1 = 40              # startup: fill the DMA head; ends as wk+xk land
W1B = 14             # fill the xq DMA wait so the clock ramp isn't reset
W1C = 6              # fill the q evac latency
W2 = 4
WDRAIN = 40          # endgame: bridge the norm staging latency

_NC = None


def _build():
    import concourse.mybir as mybir
    import concourse.tile as tile
    from concourse import bacc
    from concourse.masks import make_identity

    f32 = mybir.dt.float32
    bf16 = mybir.dt.bfloat16
    AF = mybir.ActivationFunctionType

    nc = bacc.Bacc("TRN2", target_bir_lowering=False, debug=False, num_devices=NCORES)

    qT = nc.dram_tensor("qT", [D, S], bf16, kind="ExternalInput").ap()
    kT = nc.dram_tensor("kT", [D, S], bf16, kind="ExternalInput").ap()
    vT = nc.dram_tensor("vT", [D, S], bf16, kind="ExternalInput").ap()
    wq = nc.dram_tensor("wq", [D, JC], bf16, kind="ExternalInput").ap()
    wk = nc.dram_tensor("wk", [D, JC], bf16, kind="ExternalInput").ap()
    wv = nc.dram_tensor("wv", [D, JC], bf16, kind="ExternalInput").ap()
    wo = nc.dram_tensor("wo", [JC, D], bf16, kind="ExternalInput").ap()
    # all biases in one DMA: cols 0-1 bq, 2-3 bk, 4-5 unused, 6-13 bo_eff
    bias = nc.dram_tensor("bias", [128, 14], f32, kind="ExternalInput").ap()
    out = nc.dram_tensor("out", [D, S], bf16, kind="ExternalOutput").ap()

    with tile.TileContext(nc) as tc:
        with (
            tc.tile_pool(name="const", bufs=1) as const,
            tc.tile_pool(name="inp", bufs=5) as inpool,
            tc.tile_pool(name="expp", bufs=5) as exppool,
            tc.tile_pool(name="usb", bufs=4) as usbpool,
            tc.tile_pool(name="nrm", bufs=4) as nrmpool,
            tc.tile_pool(name="osb", bufs=2) as osbpool,
            tc.tile_pool(name="psSC", bufs=2, space="PSUM") as psSC,
            tc.tile_pool(name="psOP", bufs=2, space="PSUM") as psOP,
            tc.tile_pool(name="psU", bufs=2, space="PSUM") as psU,
        ):
            # ---- weights (DMA emission order = critical path order) ----
            def load_w(ap_dram, name, n_dt, split=1):
                cols = ap_dram.shape[1]
                t = const.tile([128, n_dt * cols], bf16, tag=name)
                hd = n_dt // split
                for h in range(split):
                    nc.sync.dma_start(
                        t[:, h * hd * cols:(h + 1) * hd * cols].rearrange(
                            "p (dt j) -> p dt j", dt=hd),
                        ap_dram[h * hd * 128:(h + 1) * hd * 128, :].rearrange(
                            "(dt p) j -> p dt j", p=128),
                    )
                return [t[:, d * cols:(d + 1) * cols] for d in range(n_dt)]

            wk_sb = load_w(wk, "wk", NDT, split=2)
            b_sb = const.tile([128, 14], f32, tag="bias")
            nc.sync.dma_start(b_sb[:], bias[:])
            bq_sb, bk_sb, bo_sb = (b_sb[:, 0:2], b_sb[:, 2:4], b_sb[:, 6:14])

            def load_x(xT_dram, tb, split=1):
                xt = inpool.tile([128, NDT * TB], bf16, tag="in")
                hd = NDT // split
                for h in range(split):
                    nc.sync.dma_start(
                        xt[:, h * hd * TB:(h + 1) * hd * TB].rearrange(
                            "p (dt t) -> p dt t", dt=hd),
                        xT_dram[h * hd * 128:(h + 1) * hd * 128,
                                tb * TB:(tb + 1) * TB].rearrange(
                            "(dt p) t -> p dt t", p=128),
                    )
                return xt

            ident = const.tile([128, 128], bf16, tag="ident")
            make_identity(nc, ident[:])

            # HAM warm-up: the PE clock sits at 1.2 GHz until ~3.4us of
            # sustained activity. The input DMAs leave the PE idle for the
            # first ~8us, so burn that window on dummy transposes to hit the
            # first real matmuls at 2.4 GHz.
            _warm_n = [0]

            def warmup(n):
                for _ in range(n):
                    warm = psSC.tile(
                        [128, 128], bf16, tag="sc", name=f"warm{_warm_n[0]}")
                    _warm_n[0] += 1
                    nc.tensor.transpose(warm[:], ident[:], ident[:])

            # ---- persistent activations ----
            qpT = const.tile([128, 2 * S], bf16, tag="qpT")
            kpT = const.tile([128, 2 * S], bf16, tag="kpT")
            vp = const.tile([128, NTT * 2 * VROW], bf16, tag="vp")
            hoT = const.tile([128, 2 * S], bf16, tag="hoT")
            tmp3 = const.tile([DK, TB], bf16, tag="tmp3")

            ones_src = const.tile([128, 1], f32, tag="ones_src")
            nc.gpsimd.memset(ones_src[:], 1.0)
            ones64 = const.tile([1, DK], bf16, tag="ones64")
            nc.gpsimd.memset(ones64[:], 1.0)
            vp_ones = vp[:].rearrange(
                "p (tt seg c) -> p (tt seg) c", tt=NTT, seg=4, c=DK + 1
            )[:, :, DK:DK + 1]
            nc.vector.tensor_copy(vp_ones, ones_src[:].to_broadcast([128, NTT * 4, 1]))

            # ---- q/k projections (feature-major, one jt half at a time) ----
            _pj_n = [0]

            def proj_jt(xt, w_tiles, b_tile, dstT, tb, jt, d0=0, d1=NDT, ps_hold=[None]):
                if d0 == 0:
                    ps_hold[0] = psOP.tile(
                        [128, TB], f32, tag="mm", name=f"pj{_pj_n[0]}")
                    _pj_n[0] += 1
                ps = ps_hold[0]
                for d in range(d0, d1):
                    nc.tensor.matmul(
                        ps[:],
                        lhsT=w_tiles[d][:, jt * 128:(jt + 1) * 128],
                        rhs=xt[:, d * TB:(d + 1) * TB],
                        start=(d == 0),
                        stop=(d == NDT - 1),
                    )
                if d1 == NDT:
                    nc.vector.tensor_scalar_add(
                        dstT[:, jt * S + tb * TB: jt * S + (tb + 1) * TB],
                        ps[:],
                        b_tile[:, jt:jt + 1],
                    )

            # ---- v projection: emits [token, feature] tiles directly ----
            # out partitions = 128 tokens of tile tt, free = 256 features
            # (4 heads); evac is one strided copy into vp's 65-wide segments
            # (ones column at offset 64 pre-filled above). b_v folded into
            # b_o on the host.
            def proj_v_chunk(xt, tt):
                c = tt % 4
                vps = psOP.tile([128, JC], f32, tag="mm")
                for d in range(NDT):
                    nc.tensor.matmul(
                        vps[:],
                        lhsT=xt[:, d * TB + c * 128: d * TB + (c + 1) * 128],
                        rhs=wv_sb[d][:, :],
                        start=(d == 0),
                        stop=(d == NDT - 1),
                    )
                dst = vp[:, tt * 2 * VROW:(tt + 1) * 2 * VROW].rearrange(
                    "p (seg c) -> p seg c", seg=4, c=DK + 1)[:, :, 0:DK]
                nc.vector.tensor_copy(
                    dst, vps[:].rearrange("p (seg c) -> p seg c", seg=4, c=DK))

            # ---- attention conveyor ----
            U = {}

            def turn_scores(sb, jt, tt):
                sc = psSC.tile([128, 2 * TB], f32, tag="sc")
                for h, p0 in ((0, 0), (1, 64)):
                    nc.tensor.matmul(
                        sc[:, h * TB:(h + 1) * TB],
                        lhsT=kpT[p0:p0 + DK, jt * S + tt * 128: jt * S + (tt + 1) * 128],
                        rhs=qpT[p0:p0 + DK, jt * S + sb * TB: jt * S + (sb + 1) * TB],
                    )
                ex = exppool.tile([128, 2 * TB], bf16, tag="exp")
                nc.scalar.activation(ex[:], sc[:], AF.Exp, scale=float(1.0 / np.sqrt(DK)))
                return ex

            # mid-kernel norm: the denominator row must reach partition 0 via
            # DMA (no engine can move data across partitions; DMA can't read
            # PSUM, hence the usb staging copy which also releases psU).
            def norm_pair(sb, jt, uA, uB):
                for h, u in ((0, uA), (1, uB)):
                    usb = usbpool.tile([DK + 1, TB], f32, tag="usb")
                    nc.vector.tensor_copy(usb[:], u[:])
                    rc = nrmpool.tile([1, TB], f32, tag="rc")
                    nc.sync.dma_start(rc[:], usb[DK:DK + 1, :])
                    rc2 = nrmpool.tile([1, TB], f32, tag="rc2")
                    nc.vector.reciprocal_approx_fast(rc2[:], rc[:])
                    rb = nrmpool.tile([DK, TB], f32, tag="rb")
                    nc.gpsimd.partition_broadcast(rb[:], rc2[:])
                    if h == 0:
                        nc.vector.tensor_mul(
                            hoT[0:DK, jt * S + sb * TB: jt * S + (sb + 1) * TB],
                            usb[0:DK, :],
                            rb[:],
                        )
                    else:
                        tmp = nrmpool.tile([DK, TB], bf16, tag="tmp")
                        nc.vector.tensor_mul(tmp[:], usb[0:DK, :], rb[:])
                        nc.sync.dma_start(
                            hoT[DK:2 * DK, jt * S + sb * TB: jt * S + (sb + 1) * TB],
                            tmp[:],
                        )

            # endgame norm for pair (3,1), split in two parts so the PE can
            # run filler between them. head3 lands in a base-0 tmp3 tile (the
            # final out-proj contracts it with two K=64 matmuls), so there is
            # no partition-offset hoT DMA on the tail; the partition
            # broadcast of 1/denom runs on the (idle) PE as a K=1 outer
            # product instead of GpSimd.
            _nf = {}

            def norm_fast_p1(uA, uB):
                for h, u in ((0, uA), (1, uB)):
                    usb = usbpool.tile([DK + 1, TB], f32, tag="usb")
                    nc.vector.tensor_copy(usb[:], u[:])
                    rc = nrmpool.tile([1, TB], f32, tag="rc")
                    nc.sync.dma_start(rc[:], usb[DK:DK + 1, :])
                    _nf[h] = (usb, rc)

            def norm_fast_p2():
                rc2s = []
                for h in range(2):
                    rc2 = nrmpool.tile([1, TB], f32, tag="rc2")
                    nc.vector.reciprocal_approx_fast(rc2[:], _nf[h][1][:])
                    rc2b = nrmpool.tile([1, TB], bf16, tag="rc2b")
                    nc.scalar.copy(rc2b[:], rc2[:])
                    rc2s.append(rc2b)
                for h in range(2):
                    usb, _ = _nf[h]
                    rb = psU.tile([DK, TB], f32, tag="U", name=f"rbf{h}")
                    nc.tensor.matmul(rb[:], lhsT=ones64[:], rhs=rc2s[h][:])
                    if h == 0:
                        nc.vector.tensor_mul(
                            hoT[0:DK, S + (NTB - 1) * TB: S + NTB * TB],
                            usb[0:DK, :], rb[:])
                    else:
                        nc.vector.tensor_mul(tmp3[:], usb[0:DK, :], rb[:])

            pend = []

            def pop_pv():
                sb, jt, tt, ex = pend.pop(0)
                if tt == 0:
                    uA = psU.tile([DK + 1, TB], f32, tag="U", name=f"uA_{sb}_{jt}")
                    uB = psU.tile([DK + 1, TB], f32, tag="U", name=f"uB_{sb}_{jt}")
                    U[(sb, jt)] = (uA, uB)
                uA, uB = U[(sb, jt)]
                for h, u in ((0, uA), (1, uB)):
                    o = tt * 2 * VROW + jt * VROW + h * (DK + 1)
                    nc.tensor.matmul(
                        u[:],
                        lhsT=vp[:, o: o + DK + 1],
                        rhs=ex[:, h * TB:(h + 1) * TB],
                        start=(tt == 0),
                        stop=(tt == NTT - 1),
                    )
                if tt == NTT - 1:
                    if (sb, jt) == (NTB - 1, 1):
                        norm_fast_p1(uA, uB)
                    else:
                        norm_pair(sb, jt, uA, uB)

            def push_turn(sb, jt, tt):
                pend.append((sb, jt, tt, turn_scores(sb, jt, tt)))

            # out-proj for query block sb: 8 single-ft groups on psOP
            def outproj_ft(sb, ft, ot):
                op = psOP.tile([128, TB], f32, tag="mm")
                for jt in range(2):
                    nc.tensor.matmul(
                        op[:],
                        lhsT=wo_sb[jt][:, ft * 128:(ft + 1) * 128],
                        rhs=hoT[:, jt * S + sb * TB: jt * S + (sb + 1) * TB],
                        start=(jt == 0),
                        stop=(jt == 1),
                    )
                nc.vector.tensor_scalar_add(
                    ot[:, ft * TB:(ft + 1) * TB], op[:], bo_sb[:, ft:ft + 1]
                )
                if ft == 3 or ft == 7:
                    h0 = 0 if ft == 3 else 512
                    nc.sync.dma_start(
                        out[h0:h0 + 512, sb * TB:(sb + 1) * TB].rearrange(
                            "(ft p) t -> p ft t", p=128),
                        ot[:, (ft - 3) * TB:(ft + 1) * TB].rearrange(
                            "p (ft t) -> p ft t", ft=4),
                    )

            # last block's out-proj: pass 1 (jt0 half, during pair (3,1));
            # pass 2 contracts heads 2/3 with two K=64 matmuls so head3 can
            # stay in a base-0 tile (no partition-offset DMA on the tail).
            def outproj_p1(sb, ft, ot1):
                op = psOP.tile([128, TB], f32, tag="mm")
                nc.tensor.matmul(
                    op[:],
                    lhsT=wo_sb[0][:, ft * 128:(ft + 1) * 128],
                    rhs=hoT[:, sb * TB:(sb + 1) * TB],
                )
                nc.vector.tensor_scalar_add(
                    ot1[:, ft * TB:(ft + 1) * TB], op[:], bo_sb[:, ft:ft + 1]
                )

            def outproj_p2(sb, ft, ot1, ot):
                op = psOP.tile([128, TB], f32, tag="mm")
                nc.tensor.matmul(
                    op[:],
                    lhsT=wo_sb[1][0:64, ft * 128:(ft + 1) * 128],
                    rhs=hoT[0:64, S + sb * TB: S + (sb + 1) * TB],
                    start=True, stop=False,
                )
                nc.tensor.matmul(
                    op[:],
                    lhsT=wo3_sb[:, ft * 128:(ft + 1) * 128],
                    rhs=tmp3[:, :],
                    start=False, stop=False,
                )
                # fold the pass-1 partial in on the (idle) PE instead of a
                # Vector tensor_add: ident^T @ ot1 accumulates ot1 into op
                nc.tensor.matmul(
                    op[:],
                    lhsT=ident[:],
                    rhs=ot1[:, ft * TB:(ft + 1) * TB],
                    start=False, stop=True,
                )
                # plain copy evac, alternating engines so the tail chain is
                # half Scalar (idle after the last exp) and half Vector
                if ft % 2 == 0:
                    nc.scalar.copy(ot[:, ft * TB:(ft + 1) * TB], op[:])
                else:
                    nc.vector.tensor_copy(ot[:, ft * TB:(ft + 1) * TB], op[:])
                nc.sync.dma_start(
                    out[ft * 128:(ft + 1) * 128, sb * TB:(sb + 1) * TB],
                    ot[:, ft * TB:(ft + 1) * TB],
                )

            # ================= emission =================
            # startup: tb0 with per-jt ordering so pair (0,0) starts ASAP
            xk = load_x(kT, 0, split=4)
            wq_sb = load_w(wq, "wq", NDT, split=2)
            xq = load_x(qT, 0, split=2)
            wv_sb = load_w(wv, "wv", NDT, split=2)
            xv = load_x(vT, 0, split=2)
            warmup(W1)
            proj_jt(xk, wk_sb, bk_sb, kpT, 0, 0)
            warmup(W1B)
            proj_jt(xq, wq_sb, bq_sb, qpT, 0, 0)
            warmup(W1C)
            proj_v_chunk(xv, 0)
            proj_v_chunk(xv, 1)
            push_turn(0, 0, 0)
            push_turn(0, 0, 1)
            proj_v_chunk(xv, 2)
            proj_v_chunk(xv, 3)
            push_turn(0, 0, 2)
            push_turn(0, 0, 3)
            pop_pv()
            pop_pv()
            proj_jt(xk, wk_sb, bk_sb, kpT, 0, 1)
            proj_jt(xq, wq_sb, bq_sb, qpT, 0, 1)
            warmup(W2)
            for tb in range(1, NTB):
                xk = load_x(kT, tb)
                xv = load_x(vT, tb)
                proj_jt(xk, wk_sb, bk_sb, kpT, tb, 0)
                proj_v_chunk(xv, 4 * tb)
                proj_v_chunk(xv, 4 * tb + 1)
                push_turn(0, 0, 4 * tb)
                push_turn(0, 0, 4 * tb + 1)
                pop_pv()
                pop_pv()
                proj_jt(xk, wk_sb, bk_sb, kpT, tb, 1)
                proj_v_chunk(xv, 4 * tb + 2)
                proj_v_chunk(xv, 4 * tb + 3)
                push_turn(0, 0, 4 * tb + 2)
                push_turn(0, 0, 4 * tb + 3)
                pop_pv()
                pop_pv()
            wo_sb = load_w(wo, "wo", 2)
            wo3_sb = const.tile([DK, D], bf16, tag="wo3")
            nc.sync.dma_start(wo3_sb[:], wo[3 * DK:4 * DK, :])

            # steady conveyor over the remaining 7 pairs, two turns per
            # group: [SC SC][weave][PV PV PV PV]. The full-width weave
            # matmul after the row-tiled scores pair absorbs the PE
            # weight-load transition. Weave per group g (0..7):
            #   (s, 1) pairs: q-block s+1 projection, quarter per group 2-5
            #   (s, 0) pairs: out-proj ft g of block s-1
            #   (3, 1): out-proj pass 1 of block 3, 2 fts per group 4-7
            ot1 = const.tile([128, 8 * TB], bf16, tag="ot1")
            for sb, jt in [(0, 1)] + [(s, j) for s in range(1, NTB) for j in range(2)]:
                do_op = (jt == 0 and sb > 0)
                do_q = (jt == 1 and sb < NTB - 1)
                do_p1 = (sb, jt) == (NTB - 1, 1)
                if do_op:
                    ot = osbpool.tile([128, 8 * TB], bf16, tag="ot")
                if do_q:
                    xqs = load_x(qT, sb + 1, split=2)
                for g in range(NTT // 2):
                    push_turn(sb, jt, 2 * g)
                    push_turn(sb, jt, 2 * g + 1)
                    # early-drain discipline: each pair fully retires its own
                    # turns by its final group (the trailing PVs fit in the
                    # exp shadow), so its norm + hoT DMA are already emitted
                    # when the NEXT pair's g0 out-proj weave reads hoT, and
                    # each pair starts with an empty conveyor.
                    if g > 0:
                        pop_pv()
                        pop_pv()
                    if g == NTT // 2 - 1:
                        while pend:
                            pop_pv()
                    if do_q and g in (2, 3, 4, 5):
                        hjt, half = divmod(g - 2, 2)
                        proj_jt(xqs, wq_sb, bq_sb, qpT, sb + 1, hjt,
                                d0=half * 4, d1=half * 4 + 4)
                    if do_op:
                        outproj_ft(sb - 1, g, ot)
                    if do_p1 and g >= 4:
                        outproj_p1(NTB - 1, 2 * (g - 4), ot1)
                        outproj_p1(NTB - 1, 2 * (g - 4) + 1, ot1)
            while pend:
                pop_pv()
            # hold the PE clock up across the norm staging latency
            warmup(WDRAIN)
            norm_fast_p2()
            # final out-proj pass 2: split-K matmuls + add-evacs, per-ft DMA
            ot = osbpool.tile([128, 8 * TB], bf16, tag="ot")
            for ft in range(8):
                outproj_p2(NTB - 1, ft, ot1, ot)

    nc.compile()
    return nc


def _get_nc():
    global _NC
    if _NC is None:
        _NC = _build()
    return _NC


def make_in_maps(q, k, v, w_q, b_q, w_k, b_k, w_v, b_v, w_o, b_o):
    import ml_dtypes
    cdt = ml_dtypes.bfloat16
    q = np.asarray(q, np.float32)
    k = np.asarray(k, np.float32)
    v = np.asarray(v, np.float32)
    w_q = np.asarray(w_q, np.float32)
    w_k = np.asarray(w_k, np.float32)
    w_v = np.asarray(w_v, np.float32)
    w_o = np.asarray(w_o, np.float32)
    b_q = np.asarray(b_q, np.float32)
    b_k = np.asarray(b_k, np.float32)
    b_v = np.asarray(b_v, np.float32)
    b_o = np.asarray(b_o, np.float32)
    # v bias folded through attention (rows of P sum to 1 after norm) and
    # the out projection: exact for any inputs.
    b_o_eff = b_o + w_o.T @ b_v

    in_maps = []
    for c in range(NCORES):
        b, g = divmod(c, GROUPS)
        js = slice(g * JC, (g + 1) * JC)
        bias2 = lambda x: x[js].reshape(2, 128).T
        bo2 = (b_o_eff.reshape(8, 128).T if g == 0
               else np.zeros((128, 8), np.float32))
        bias_all = np.concatenate(
            [bias2(b_q), bias2(b_k), np.zeros((128, 2), np.float32), bo2], axis=1)
        in_maps.append({
            "qT": np.ascontiguousarray(q[b].T).astype(cdt),
            "kT": np.ascontiguousarray(k[b].T).astype(cdt),
            "vT": np.ascontiguousarray(v[b].T).astype(cdt),
            "wq": np.ascontiguousarray(w_q[:, js]).astype(cdt),
            "wk": np.ascontiguousarray(w_k[:, js]).astype(cdt),
            "wv": np.ascontiguousarray(w_v[:, js]).astype(cdt),
            "wo": np.ascontiguousarray(w_o[js, :]).astype(cdt),
            "bias": np.ascontiguousarray(bias_all, dtype=np.float32),
        })
    return in_maps


def gather(results):
    out = np.zeros((B, S, D), np.float32)
    for c in range(NCORES):
        b = c // GROUPS
        out[b] += results[c]["out"].T.astype(np.float32)
    return out


def kernel(q, k, v, w_q, b_q, w_k, b_k, w_v, b_v, w_o, b_o, _trace=False):
    from concourse.bass_utils import run_bass_kernel_spmd

    nc = _get_nc()
    in_maps = make_in_maps(q, k, v, w_q, b_q, w_k, b_k, w_v, b_v, w_o, b_o)
    res = run_bass_kernel_spmd(nc, in_maps, core_ids=list(range(NCORES)), trace=_trace)
    out = gather(res.results)
    if _trace:
        kernel.last_exec_time_ns = res.exec_time_ns
        kernel.last_results = res
    return out


# revision 21
# speedup vs baseline: 1.0007x; 1.0007x over previous
"""Multi-head attention (B=2, S=2048, D=1024, H=16) on 8 TRN2 NeuronCores.

Sharding: data-parallel over batch (2 groups of 4 cores), tensor-parallel over
heads within a group (4 heads = 256 feature columns per core). Each core:
  - projects its batch's q/k/v (full D contraction) into its 256-col head slice
  - runs full attention for its 4 heads over the 2048-token sequence
  - applies its 256-row slice of w_o, producing a partial [D, S] output (bf16)
Host sums the 4 partials per batch (+ b_o folded into one core per batch) and
transposes back to [S, D].

Structure (226us baseline -> this version):
  - flat conveyor of per-tt "turns" (scores -> exp -> P@V) across all 8
    (sb, jt) pairs, grouped TWO turns at a time: [SC SC][weave][PV PV PV PV].
    Scores pairs adjacent halves amortize the row-tiled weight-load
    serialization; the full-width weave matmul after them eats the
    transition cost while doing useful work.
  - v-projection emits P@V-ready [token, feature] tiles directly
    (lhsT=x chunk, rhs=w_v tile) -- no PE transposes, no vector repack
    copies. b_v is folded into b_o on the host (b_o_eff = b_o + W_o^T b_v,
    exact), so the v evac is a single strided copy.
  - endgame: the last pair's softmax norm reads PSUM directly (no usb copy,
    no rc DMA), head3 lands in a base-0 tmp tile and the final out-proj
    contracts it with a separately-loaded w_o row-slice (2x K=64 matmuls),
    avoiding the 3us SBUF->SBUF partition-offset DMA; final output DMA is
    split per-ft across 8 queues; a few identity transposes keep the PE
    clock up across the norm latency.
  - mid-kernel psU release copies run on GpSimd (idle) instead of Vector.
Softmax denominator comes from a ones-column appended to each head's V tile
(PSUM-accumulated by the P@V matmul).
"""

import numpy as np

B, S, D, H = 2, 2048, 1024, 16
DK = D // H          # 64
NCORES = 8
GROUPS = 4           # head-groups (cores) per batch
JC = D // GROUPS     # 256 feature columns per core (4 heads)
TB = 512             # token block (matmul moving free dim)
NTB = S // TB        # 4
NDT = D // 128       # 8 contraction tiles for projections
NTT = S // 128       # 16 key-token tiles per sequence
VROW = 2 * (DK + 1)  # 130: per-jt vp row segment (2 heads x (64 v cols + ones))

# Warmup transposes do NOT ramp the PE clock (the HAM ramp tracks ~3.4us of
# sustained real matmul work); they only prevent the idle gate-down (~2us of
# idle drops the clock back to 1.2 GHz). So: almost none at startup (the head
# is DMA-bound at low clock regardless), many across the endgame norm-chain
# latency where the clock is up and must stay up.
W1 = 40              # startup: fill the DMA head; ends as wk+xk land
W1B = 14             # fill the xq DMA wait so the clock ramp isn't reset
W1C = 6              # fill the q evac latency
W2 = 4
WDRAIN = 40          # endgame: bridge the norm staging latency

_NC = None


def _build():
    import concourse.mybir as mybir
    import concourse.tile as tile
    from concourse import bacc
    from concourse.masks import make_identity

    f32 = mybir.dt.float32
    bf16 = mybir.dt.bfloat16
    AF = mybir.ActivationFunctionType

    nc = bacc.Bacc("TRN2", target_bir_lowering=False, debug=False, num_devices=NCORES)

    qT = nc.dram_tensor("qT", [D, S], bf16, kind="ExternalInput").ap()
    kT = nc.dram_tensor("kT", [D, S], bf16, kind="ExternalInput").ap()
    vT = nc.dram_tensor("vT", [D, S], bf16, kind="ExternalInput").ap()
    wq = nc.dram_tensor("wq", [D, JC], bf16, kind="ExternalInput").ap()
    wk = nc.dram_tensor("wk", [D, JC], bf16, kind="ExternalInput").ap()
    wv = nc.dram_tensor("wv", [D, JC], bf16, kind="ExternalInput").ap()
    wo = nc.dram_tensor("wo", [JC, D], bf16, kind="ExternalInput").ap()
    # all biases in one DMA: cols 0-1 bq, 2-3 bk, 4-5 unused, 6-13 bo_eff
    bias = nc.dram_tensor("bias", [128, 14], f32, kind="ExternalInput").ap()
    out = nc.dram_tensor("out", [D, S], bf16, kind="ExternalOutput").ap()

    with tile.TileContext(nc) as tc:
        with (
            tc.tile_pool(name="const", bufs=1) as const,
            tc.tile_pool(name="inp", bufs=5) as inpool,
            tc.tile_pool(name="expp", bufs=5) as exppool,
            tc.tile_pool(name="usb", bufs=4) as usbpool,
            tc.tile_pool(name="nrm", bufs=4) as nrmpool,
            tc.tile_pool(name="osb", bufs=2) as osbpool,
            tc.tile_pool(name="psSC", bufs=2, space="PSUM") as psSC,
            tc.tile_pool(name="psOP", bufs=2, space="PSUM") as psOP,
            tc.tile_pool(name="psU", bufs=2, space="PSUM") as psU,
        ):
            # ---- weights (DMA emission order = critical path order) ----
            def load_w(ap_dram, name, n_dt, split=1):
                cols = ap_dram.shape[1]
                t = const.tile([128, n_dt * cols], bf16, tag=name)
                hd = n_dt // split
                for h in range(split):
                    nc.sync.dma_start(
                        t[:, h * hd * cols:(h + 1) * hd * cols].rearrange(
                            "p (dt j) -> p dt j", dt=hd),
                        ap_dram[h * hd * 128:(h + 1) * hd * 128, :].rearrange(
                            "(dt p) j -> p dt j", p=128),
                    )
                return [t[:, d * cols:(d + 1) * cols] for d in range(n_dt)]

            wk_sb = load_w(wk, "wk", NDT, split=2)
            b_sb = const.tile([128, 14], f32, tag="bias")
            nc.sync.dma_start(b_sb[:], bias[:])
            bq_sb, bk_sb, bo_sb = (b_sb[:, 0:2], b_sb[:, 2:4], b_sb[:, 6:14])

            def load_x(xT_dram, tb, split=1):
                xt = inpool.tile([128, NDT * TB], bf16, tag="in")
                hd = NDT // split
                for h in range(split):
                    nc.sync.dma_start(
                        xt[:, h * hd * TB:(h + 1) * hd * TB].rearrange(
                            "p (dt t) -> p dt t", dt=hd),
                        xT_dram[h * hd * 128:(h + 1) * hd * 128,
                                tb * TB:(tb + 1) * TB].rearrange(
                            "(dt p) t -> p dt t", p=128),
                    )
                return xt

            ident = const.tile([128, 128], bf16, tag="ident")
            make_identity(nc, ident[:])

            # HAM warm-up: the PE clock sits at 1.2 GHz until ~3.4us of
            # sustained activity. The input DMAs leave the PE idle for the
            # first ~8us, so burn that window on dummy transposes to hit the
            # first real matmuls at 2.4 GHz.
            _warm_n = [0]

            def warmup(n):
                for _ in range(n):
                    warm = psSC.tile(
                        [128, 128], bf16, tag="sc", name=f"warm{_warm_n[0]}")
                    _warm_n[0] += 1
                    nc.tensor.transpose(warm[:], ident[:], ident[:])

            # ---- persistent activations ----
            qpT = const.tile([128, 2 * S], bf16, tag="qpT")
            kpT = const.tile([128, 2 * S], bf16, tag="kpT")
            vp = const.tile([128, NTT * 2 * VROW], bf16, tag="vp")
            hoT = const.tile([128, 2 * S], bf16, tag="hoT")
            tmp3 = const.tile([DK, TB], bf16, tag="tmp3")

            ones_src = const.tile([128, 1], f32, tag="ones_src")
            nc.gpsimd.memset(ones_src[:], 1.0)
            ones64 = const.tile([1, DK], bf16, tag="ones64")
            nc.gpsimd.memset(ones64[:], 1.0)
            vp_ones = vp[:].rearrange(
                "p (tt seg c) -> p (tt seg) c", tt=NTT, seg=4, c=DK + 1
            )[:, :, DK:DK + 1]
            nc.vector.tensor_copy(vp_ones, ones_src[:].to_broadcast([128, NTT * 4, 1]))

            # ---- q/k projections (feature-major, one jt half at a time) ----
            _pj_n = [0]

            def proj_jt(xt, w_tiles, b_tile, dstT, tb, jt, d0=0, d1=NDT, ps_hold=[None]):
                if d0 == 0:
                    ps_hold[0] = psOP.tile(
                        [128, TB], f32, tag="mm", name=f"pj{_pj_n[0]}")
                    _pj_n[0] += 1
                ps = ps_hold[0]
                for d in range(d0, d1):
                    nc.tensor.matmul(
                        ps[:],
                        lhsT=w_tiles[d][:, jt * 128:(jt + 1) * 128],
                        rhs=xt[:, d * TB:(d + 1) * TB],
                        start=(d == 0),
                        stop=(d == NDT - 1),
                    )
                if d1 == NDT:
                    nc.vector.tensor_scalar_add(
                        dstT[:, jt * S + tb * TB: jt * S + (tb + 1) * TB],
                        ps[:],
                        b_tile[:, jt:jt + 1],
                    )

            # ---- v projection: emits [token, feature] tiles directly ----
            # out partitions = 128 tokens of tile tt, free = 256 features
            # (4 heads); evac is one strided copy into vp's 65-wide segments
            # (ones column at offset 64 pre-filled above). b_v folded into
            # b_o on the host.
            def proj_v_chunk(xt, tt):
                c = tt % 4
                vps = psOP.tile([128, JC], f32, tag="mm")
                for d in range(NDT):
                    nc.tensor.matmul(
                        vps[:],
                        lhsT=xt[:, d * TB + c * 128: d * TB + (c + 1) * 128],
                        rhs=wv_sb[d][:, :],
                        start=(d == 0),
                        stop=(d == NDT - 1),
                    )
                dst = vp[:, tt * 2 * VROW:(tt + 1) * 2 * VROW].rearrange(
                    "p (seg c) -> p seg c", seg=4, c=DK + 1)[:, :, 0:DK]
                nc.vector.tensor_copy(
                    dst, vps[:].rearrange("p (seg c) -> p seg c", seg=4, c=DK))

            # ---- attention conveyor ----
            U = {}

            def turn_scores(sb, jt, tt):
                sc = psSC.tile([128, 2 * TB], f32, tag="sc")
                for h, p0 in ((0, 0), (1, 64)):
                    nc.tensor.matmul(
                        sc[:, h * TB:(h + 1) * TB],
                        lhsT=kpT[p0:p0 + DK, jt * S + tt * 128: jt * S + (tt + 1) * 128],
                        rhs=qpT[p0:p0 + DK, jt * S + sb * TB: jt * S + (sb + 1) * TB],
                    )
                ex = exppool.tile([128, 2 * TB], bf16, tag="exp")
                nc.scalar.activation(ex[:], sc[:], AF.Exp, scale=float(1.0 / np.sqrt(DK)))
                return ex

            # mid-kernel norm: the denominator row must reach partition 0 via
            # DMA (no engine can move data across partitions; DMA can't read
            # PSUM, hence the usb staging copy which also releases psU).
            def norm_pair(sb, jt, uA, uB):
                for h, u in ((0, uA), (1, uB)):
                    usb = usbpool.tile([DK + 1, TB], f32, tag="usb")
                    nc.vector.tensor_copy(usb[:], u[:])
                    rc = nrmpool.tile([1, TB], f32, tag="rc")
                    nc.sync.dma_start(rc[:], usb[DK:DK + 1, :])
                    rc2 = nrmpool.tile([1, TB], f32, tag="rc2")
                    nc.vector.reciprocal_approx_fast(rc2[:], rc[:])
                    rb = nrmpool.tile([DK, TB], f32, tag="rb")
                    nc.gpsimd.partition_broadcast(rb[:], rc2[:])
                    if h == 0:
                        nc.vector.tensor_mul(
                            hoT[0:DK, jt * S + sb * TB: jt * S + (sb + 1) * TB],
                            usb[0:DK, :],
                            rb[:],
                        )
                    else:
                        tmp = nrmpool.tile([DK, TB], bf16, tag="tmp")
                        nc.vector.tensor_mul(tmp[:], usb[0:DK, :], rb[:])
                        nc.sync.dma_start(
                            hoT[DK:2 * DK, jt * S + sb * TB: jt * S + (sb + 1) * TB],
                            tmp[:],
                        )

            # endgame norm for pair (3,1), split in two parts so the PE can
            # run filler between them. head3 lands in a base-0 tmp3 tile (the
            # final out-proj contracts it with two K=64 matmuls), so there is
            # no partition-offset hoT DMA on the tail; the partition
            # broadcast of 1/denom runs on the (idle) PE as a K=1 outer
            # product instead of GpSimd.
            _nf = {}

            def norm_fast_p1(uA, uB):
                for h, u in ((0, uA), (1, uB)):
                    usb = usbpool.tile([DK + 1, TB], f32, tag="usb")
                    nc.vector.tensor_copy(usb[:], u[:])
                    rc = nrmpool.tile([1, TB], f32, tag="rc")
                    nc.sync.dma_start(rc[:], usb[DK:DK + 1, :])
                    _nf[h] = (usb, rc)

            def norm_fast_p2():
                rc2s = []
                for h in range(2):
                    rc2 = nrmpool.tile([1, TB], f32, tag="rc2")
                    nc.vector.reciprocal_approx_fast(rc2[:], _nf[h][1][:])
                    rc2b = nrmpool.tile([1, TB], bf16, tag="rc2b")
                    nc.scalar.copy(rc2b[:], rc2[:])
                    rc2s.append(rc2b)
                for h in range(2):
                    usb, _ = _nf[h]
                    rb = psU.tile([DK, TB], f32, tag="U", name=f"rbf{h}")
                    nc.tensor.matmul(rb[:], lhsT=ones64[:], rhs=rc2s[h][:])
                    if h == 0:
                        nc.vector.tensor_mul(
                            hoT[0:DK, S + (NTB - 1) * TB: S + NTB * TB],
                            usb[0:DK, :], rb[:])
                    else:
                        nc.vector.tensor_mul(tmp3[:], usb[0:DK, :], rb[:])

            pend = []

            def pop_pv():
                sb, jt, tt, ex = pend.pop(0)
                if tt == 0:
                    uA = psU.tile([DK + 1, TB], f32, tag="U", name=f"uA_{sb}_{jt}")
                    uB = psU.tile([DK + 1, TB], f32, tag="U", name=f"uB_{sb}_{jt}")
                    U[(sb, jt)] = (uA, uB)
                uA, uB = U[(sb, jt)]
                for h, u in ((0, uA), (1, uB)):
                    o = tt * 2 * VROW + jt * VROW + h * (DK + 1)
                    nc.tensor.matmul(
                        u[:],
                        lhsT=vp[:, o: o + DK + 1],
                        rhs=ex[:, h * TB:(h + 1) * TB],
                        start=(tt == 0),
                        stop=(tt == NTT - 1),
                    )
                if tt == NTT - 1:
                    if (sb, jt) == (NTB - 1, 1):
                        norm_fast_p1(uA, uB)
                    else:
                        norm_pair(sb, jt, uA, uB)

            def push_turn(sb, jt, tt):
                pend.append((sb, jt, tt, turn_scores(sb, jt, tt)))

            # out-proj for query block sb: 8 single-ft groups on psOP
            def outproj_ft(sb, ft, ot):
                op = psOP.tile([128, TB], f32, tag="mm")
                for jt in range(2):
                    nc.tensor.matmul(
                        op[:],
                        lhsT=wo_sb[jt][:, ft * 128:(ft + 1) * 128],
                        rhs=hoT[:, jt * S + sb * TB: jt * S + (sb + 1) * TB],
                        start=(jt == 0),
                        stop=(jt == 1),
                    )
                nc.vector.tensor_scalar_add(
                    ot[:, ft * TB:(ft + 1) * TB], op[:], bo_sb[:, ft:ft + 1]
                )
                if ft == 3 or ft == 7:
                    h0 = 0 if ft == 3 else 512
                    nc.sync.dma_start(
                        out[h0:h0 + 512, sb * TB:(sb + 1) * TB].rearrange(
                            "(ft p) t -> p ft t", p=128),
                        ot[:, (ft - 3) * TB:(ft + 1) * TB].rearrange(
                            "p (ft t) -> p ft t", ft=4),
                    )

            # last block's out-proj: pass 1 (jt0 half, during pair (3,1));
            # pass 2 contracts heads 2/3 with two K=64 matmuls so head3 can
            # stay in a base-0 tile (no partition-offset DMA on the tail).
            def outproj_p1(sb, ft, ot1):
                op = psOP.tile([128, TB], f32, tag="mm")
                nc.tensor.matmul(
                    op[:],
                    lhsT=wo_sb[0][:, ft * 128:(ft + 1) * 128],
                    rhs=hoT[:, sb * TB:(sb + 1) * TB],
                )
                nc.vector.tensor_scalar_add(
                    ot1[:, ft * TB:(ft + 1) * TB], op[:], bo_sb[:, ft:ft + 1]
                )

            def outproj_p2(sb, ft, ot1, ot):
                op = psOP.tile([128, TB], f32, tag="mm")
                nc.tensor.matmul(
                    op[:],
                    lhsT=wo_sb[1][0:64, ft * 128:(ft + 1) * 128],
                    rhs=hoT[0:64, S + sb * TB: S + (sb + 1) * TB],
                    start=True, stop=False,
                )
                nc.tensor.matmul(
                    op[:],
                    lhsT=wo3_sb[:, ft * 128:(ft + 1) * 128],
                    rhs=tmp3[:, :],
                    start=False, stop=False,
                )
                # fold the pass-1 partial in on the (idle) PE instead of a
                # Vector tensor_add: ident^T @ ot1 accumulates ot1 into op
                nc.tensor.matmul(
                    op[:],
                    lhsT=ident[:],
                    rhs=ot1[:, ft * TB:(ft + 1) * TB],
                    start=False, stop=True,
                )
                # plain copy evac, alternating engines so the tail chain is
                # half Scalar (idle after the last exp) and half Vector
                if ft % 2 == 0:
                    nc.scalar.copy(ot[:, ft * TB:(ft + 1) * TB], op[:])
                else:
                    nc.vector.tensor_copy(ot[:, ft * TB:(ft + 1) * TB], op[:])
                nc.sync.dma_start(
                    out[ft * 128:(ft + 1) * 128, sb * TB:(sb + 1) * TB],
                    ot[:, ft * TB:(ft + 1) * TB],
                )

            # ================= emission =================
            # startup: tb0 with per-jt ordering so pair (0,0) starts ASAP
            xk = load_x(kT, 0, split=4)
            wq_sb = load_w(wq, "wq", NDT, split=2)
            xq = load_x(qT, 0, split=2)
            wv_sb = load_w(wv, "wv", NDT, split=2)
            xv = load_x(vT, 0, split=2)
            warmup(W1)
            proj_jt(xk, wk_sb, bk_sb, kpT, 0, 0)
            warmup(W1B)
            proj_jt(xq, wq_sb, bq_sb, qpT, 0, 0)
            warmup(W1C)
            proj_v_chunk(xv, 0)
            proj_v_chunk(xv, 1)
            push_turn(0, 0, 0)
            push_turn(0, 0, 1)
            proj_v_chunk(xv, 2)
            proj_v_chunk(xv, 3)
            push_turn(0, 0, 2)
            push_turn(0, 0, 3)
            pop_pv()
            pop_pv()
            proj_jt(xk, wk_sb, bk_sb, kpT, 0, 1)
            proj_jt(xq, wq_sb, bq_sb, qpT, 0, 1)
            warmup(W2)
            for tb in range(1, NTB):
                xk = load_x(kT, tb)
                xv = load_x(vT, tb)
                proj_jt(xk, wk_sb, bk_sb, kpT, tb, 0)
                proj_v_chunk(xv, 4 * tb)
                proj_v_chunk(xv, 4 * tb + 1)
                push_turn(0, 0, 4 * tb)
                push_turn(0, 0, 4 * tb + 1)
                pop_pv()
                pop_pv()
                proj_jt(xk, wk_sb, bk_sb, kpT, tb, 1)
                proj_v_chunk(xv, 4 * tb + 2)
                proj_v_chunk(xv, 4 * tb + 3)
                push_turn(0, 0, 4 * tb + 2)
                push_turn(0, 0, 4 * tb + 3)
                pop_pv()
                pop_pv()
            wo_sb = load_w(wo, "wo", 2)
            wo3_sb = const.tile([DK, D], bf16, tag="wo3")
            nc.sync.dma_start(wo3_sb[:], wo[3 * DK:4 * DK, :])

            # steady conveyor over the remaining 7 pairs, two turns per
            # group: [SC SC][weave][PV PV PV PV]. The full-width weave
            # matmul after the row-tiled scores pair absorbs the PE
            # weight-load transition. Weave per group g (0..7):
            #   (s, 1) pairs: q-block s+1 projection, quarter per group 2-5
            #   (s, 0) pairs: out-proj ft g of block s-1
            #   (3, 1): out-proj pass 1 of block 3, 2 fts per group 4-7
            ot1 = const.tile([128, 8 * TB], bf16, tag="ot1")
            for sb, jt in [(0, 1)] + [(s, j) for s in range(1, NTB) for j in range(2)]:
                do_op = (jt == 0 and sb > 0)
                do_q = (jt == 1 and sb < NTB - 1)
                do_p1 = (sb, jt) == (NTB - 1, 1)
                if do_op:
                    ot = osbpool.tile([128, 8 * TB], bf16, tag="ot")
                if do_q:
                    xqs = load_x(qT, sb + 1, split=2)
                for g in range(NTT // 2):
                    push_turn(sb, jt, 2 * g)
                    push_turn(sb, jt, 2 * g + 1)
                    # pops FIRST: group 0's pops emit the previous pair's
                    # norm, which the ft-0 out-proj weave reads (deps are
                    # tracked in emission order)
                    pop_pv()
                    pop_pv()
                    if do_p1 and g == NTT // 2 - 1:
                        # drain the last pair's final turns now: their PVs
                        # fit in the exp shadow and the norm staging starts
                        # two turns earlier
                        pop_pv()
                        pop_pv()
                    if do_q and g in (2, 3, 4, 5):
                        hjt, half = divmod(g - 2, 2)
                        proj_jt(xqs, wq_sb, bq_sb, qpT, sb + 1, hjt,
                                d0=half * 4, d1=half * 4 + 4)
                    if do_op:
                        outproj_ft(sb - 1, g, ot)
                    if do_p1 and g >= 4:
                        outproj_p1(NTB - 1, 2 * (g - 4), ot1)
                        outproj_p1(NTB - 1, 2 * (g - 4) + 1, ot1)
            while pend:
                pop_pv()
            # hold the PE clock up across the norm staging latency
            warmup(WDRAIN)
            norm_fast_p2()
            # final out-proj pass 2: split-K matmuls + add-evacs, per-ft DMA
            ot = osbpool.tile([128, 8 * TB], bf16, tag="ot")
            for ft in range(8):
                outproj_p2(NTB - 1, ft, ot1, ot)

    nc.compile()
    return nc


def _get_nc():
    global _NC
    if _NC is None:
        _NC = _build()
    return _NC


def make_in_maps(q, k, v, w_q, b_q, w_k, b_k, w_v, b_v, w_o, b_o):
    import ml_dtypes
    cdt = ml_dtypes.bfloat16
    q = np.asarray(q, np.float32)
    k = np.asarray(k, np.float32)
    v = np.asarray(v, np.float32)
    w_q = np.asarray(w_q, np.float32)
    w_k = np.asarray(w_k, np.float32)
    w_v = np.asarray(w_v, np.float32)
    w_o = np.asarray(w_o, np.float32)
    b_q = np.asarray(b_q, np.float32)
    b_k = np.asarray(b_k, np.float32)
    b_v = np.asarray(b_v, np.float32)
    b_o = np.asarray(b_o, np.float32)
    # v bias folded through attention (rows of P sum to 1 after norm) and
    # the out projection: exact for any inputs.
    b_o_eff = b_o + w_o.T @ b_v

    in_maps = []
    for c in range(NCORES):
        b, g = divmod(c, GROUPS)
        js = slice(g * JC, (g + 1) * JC)
        bias2 = lambda x: x[js].reshape(2, 128).T
        bo2 = (b_o_eff.reshape(8, 128).T if g == 0
               else np.zeros((128, 8), np.float32))
        bias_all = np.concatenate(
            [bias2(b_q), bias2(b_k), np.zeros((128, 2), np.float32), bo2], axis=1)
        in_maps.append({
            "qT": np.ascontiguousarray(q[b].T).astype(cdt),
            "kT": np.ascontiguousarray(k[b].T).astype(cdt),
            "vT": np.ascontiguousarray(v[b].T).astype(cdt),
            "wq": np.ascontiguousarray(w_q[:, js]).astype(cdt),
            "wk": np.ascontiguousarray(w_k[:, js]).astype(cdt),
            "wv": np.ascontiguousarray(w_v[:, js]).astype(cdt),
            "wo": np.ascontiguousarray(w_o[js, :]).astype(cdt),
            "bias": np.ascontiguousarray(bias_all, dtype=np.float32),
        })
    return in_maps


def gather(results):
    out = np.zeros((B, S, D), np.float32)
    for c in range(NCORES):
        b = c // GROUPS
        out[b] += results[c]["out"].T.astype(np.float32)
    return out


def kernel(q, k, v, w_q, b_q, w_k, b_k, w_v, b_v, w_o, b_o, _trace=False):
    from concourse.bass_utils import run_bass_kernel_spmd

    nc = _get_nc()
    in_maps = make_in_maps(q, k, v, w_q, b_q, w_k, b_k, w_v, b_v, w_o, b_o)
    res = run_bass_kernel_spmd(nc, in_maps, core_ids=list(range(NCORES)), trace=_trace)
    out = gather(res.results)
    if _trace:
        kernel.last_exec_time_ns = res.exec_time_ns
        kernel.last_results = res
    return out


# revision 23
# speedup vs baseline: 1.0296x; 1.0290x over previous
"""Multi-head attention (B=2, S=2048, D=1024, H=16) on 8 TRN2 NeuronCores.

Sharding: data-parallel over batch (2 groups of 4 cores), tensor-parallel over
heads within a group (4 heads = 256 feature columns per core). Each core:
  - projects its batch's q/k/v (full D contraction) into its 256-col head slice
  - runs full attention for its 4 heads over the 2048-token sequence
  - applies its 256-row slice of w_o, producing a partial [D, S] output (bf16)
Host sums the 4 partials per batch (+ b_o folded into one core per batch) and
transposes back to [S, D].

Structure (226us baseline -> ~213us measured):
  - flat conveyor of per-tt "turns" (scores -> exp -> P@V) across all 8
    (sb, jt) pairs, grouped TWO turns at a time: [SC SC][PV PV PV PV][weave],
    software-pipelined with lag 2. The steady stretches run at the ScalarE
    exp rate (~1us per [128,1024] tile); the projection stretch and weave
    groups are PE-bound.
  - v-projection emits P@V-ready [token, feature] tiles directly
    (lhsT=x chunk, rhs=w_v tile) -- no PE transposes, no vector repack
    copies. b_v is folded into b_o on the host (b_o_eff = b_o + W_o^T b_v,
    exact), so the v evac is a single strided copy.
  - endgame: the last pair drains two turns early (its PVs fit in the exp
    shadow); head3 of its norm lands in a base-0 tmp tile and the final
    out-proj contracts it with a separately-loaded w_o row-slice (2x K=64
    matmuls), avoiding a 3us SBUF->SBUF partition-offset DMA; 1/denom is
    partition-broadcast by a K=1 PE outer product (bf16, converted on the
    idle ScalarE); the final output DMA goes per-ft across queues; WDRAIN
    identity transposes keep the PE clock from idle-gating across the norm
    staging latency (the gate trips after ~2us idle and costs a 3.4us
    re-ramp at half clock).
Softmax denominator comes from a ones-column appended to each head's V tile
(PSUM-accumulated by the P@V matmul). The denominator row must reach
partition 0 by DMA: engines cannot move data across partitions, and DMA
cannot read PSUM, hence the usb staging copy (which also releases psU).
"""

import numpy as np

B, S, D, H = 2, 2048, 1024, 16
DK = D // H          # 64
NCORES = 8
GROUPS = 4           # head-groups (cores) per batch
JC = D // GROUPS     # 256 feature columns per core (4 heads)
TB = 512             # token block (matmul moving free dim)
NTB = S // TB        # 4
NDT = D // 128       # 8 contraction tiles for projections
NTT = S // 128       # 16 key-token tiles per sequence
VROW = 2 * (DK + 1)  # 130: per-jt vp row segment (2 heads x (64 v cols + ones))

# Warmup transposes do NOT ramp the PE clock (the HAM ramp tracks ~3.4us of
# sustained real matmul work); they only prevent the idle gate-down (~2us of
# idle drops the clock back to 1.2 GHz). So: almost none at startup (the head
# is DMA-bound at low clock regardless), many across the endgame norm-chain
# latency where the clock is up and must stay up.
W1 = 40              # startup: fill the DMA head; ends as wk+xk land
W1B = 14             # fill the xq DMA wait so the clock ramp isn't reset
W1C = 6              # fill the q evac latency
W2 = 4
WDRAIN = 40          # endgame: bridge the norm staging latency

_NC = None


def _build():
    import concourse.mybir as mybir
    import concourse.tile as tile
    from concourse import bacc
    from concourse.masks import make_identity

    f32 = mybir.dt.float32
    bf16 = mybir.dt.bfloat16
    AF = mybir.ActivationFunctionType

    nc = bacc.Bacc("TRN2", target_bir_lowering=False, debug=False, num_devices=NCORES)

    qT = nc.dram_tensor("qT", [D, S], bf16, kind="ExternalInput").ap()
    kT = nc.dram_tensor("kT", [D, S], bf16, kind="ExternalInput").ap()
    vT = nc.dram_tensor("vT", [D, S], bf16, kind="ExternalInput").ap()
    wq = nc.dram_tensor("wq", [D, JC], bf16, kind="ExternalInput").ap()
    wk = nc.dram_tensor("wk", [D, JC], bf16, kind="ExternalInput").ap()
    wv = nc.dram_tensor("wv", [D, JC], bf16, kind="ExternalInput").ap()
    wo = nc.dram_tensor("wo", [JC, D], bf16, kind="ExternalInput").ap()
    # all biases in one DMA: cols 0-1 bq, 2-3 bk, 4-5 unused, 6-13 bo_eff
    bias = nc.dram_tensor("bias", [128, 14], f32, kind="ExternalInput").ap()
    out = nc.dram_tensor("out", [D, S], bf16, kind="ExternalOutput").ap()

    with tile.TileContext(nc) as tc:
        with (
            tc.tile_pool(name="const", bufs=1) as const,
            tc.tile_pool(name="inp", bufs=5) as inpool,
            tc.tile_pool(name="expp", bufs=4) as exppool,
            tc.tile_pool(name="usb", bufs=4) as usbpool,
            tc.tile_pool(name="nrm", bufs=4) as nrmpool,
            tc.tile_pool(name="osb", bufs=2) as osbpool,
            tc.tile_pool(name="psSC", bufs=2, space="PSUM") as psSC,
            tc.tile_pool(name="psOP", bufs=2, space="PSUM") as psOP,
            tc.tile_pool(name="psU", bufs=2, space="PSUM") as psU,
        ):
            # ---- weights (DMA emission order = critical path order) ----
            def load_w(ap_dram, name, n_dt, split=1):
                cols = ap_dram.shape[1]
                t = const.tile([128, n_dt * cols], bf16, tag=name)
                hd = n_dt // split
                for h in range(split):
                    nc.sync.dma_start(
                        t[:, h * hd * cols:(h + 1) * hd * cols].rearrange(
                            "p (dt j) -> p dt j", dt=hd),
                        ap_dram[h * hd * 128:(h + 1) * hd * 128, :].rearrange(
                            "(dt p) j -> p dt j", p=128),
                    )
                return [t[:, d * cols:(d + 1) * cols] for d in range(n_dt)]

            wk_sb = load_w(wk, "wk", NDT, split=2)
            b_sb = const.tile([128, 14], f32, tag="bias")
            nc.sync.dma_start(b_sb[:], bias[:])
            bq_sb, bk_sb, bo_sb = (b_sb[:, 0:2], b_sb[:, 2:4], b_sb[:, 6:14])

            def load_x(xT_dram, tb, split=1):
                xt = inpool.tile([128, NDT * TB], bf16, tag="in")
                hd = NDT // split
                for h in range(split):
                    nc.sync.dma_start(
                        xt[:, h * hd * TB:(h + 1) * hd * TB].rearrange(
                            "p (dt t) -> p dt t", dt=hd),
                        xT_dram[h * hd * 128:(h + 1) * hd * 128,
                                tb * TB:(tb + 1) * TB].rearrange(
                            "(dt p) t -> p dt t", p=128),
                    )
                return xt

            ident = const.tile([128, 128], bf16, tag="ident")
            make_identity(nc, ident[:])

            # HAM warm-up: the PE clock sits at 1.2 GHz until ~3.4us of
            # sustained activity. The input DMAs leave the PE idle for the
            # first ~8us, so burn that window on dummy transposes to hit the
            # first real matmuls at 2.4 GHz.
            _warm_n = [0]

            def warmup(n):
                for _ in range(n):
                    warm = psSC.tile(
                        [128, 128], bf16, tag="sc", name=f"warm{_warm_n[0]}")
                    _warm_n[0] += 1
                    nc.tensor.transpose(warm[:], ident[:], ident[:])

            # ---- persistent activations ----
            qpT = const.tile([128, 2 * S], bf16, tag="qpT")
            kpT = const.tile([128, 2 * S], bf16, tag="kpT")
            vp = const.tile([128, NTT * 2 * VROW], bf16, tag="vp")
            hoT = const.tile([128, 2 * S], bf16, tag="hoT")
            tmp3 = const.tile([DK, TB], bf16, tag="tmp3")

            ones_src = const.tile([128, 1], f32, tag="ones_src")
            nc.gpsimd.memset(ones_src[:], 1.0)
            ones64 = const.tile([1, DK], bf16, tag="ones64")
            nc.gpsimd.memset(ones64[:], 1.0)
            vp_ones = vp[:].rearrange(
                "p (tt seg c) -> p (tt seg) c", tt=NTT, seg=4, c=DK + 1
            )[:, :, DK:DK + 1]
            nc.vector.tensor_copy(vp_ones, ones_src[:].to_broadcast([128, NTT * 4, 1]))

            # ---- q/k projections (feature-major, one jt half at a time) ----
            _pj_n = [0]

            def proj_jt(xt, w_tiles, b_tile, dstT, tb, jt, d0=0, d1=NDT, ps_hold=[None]):
                if d0 == 0:
                    ps_hold[0] = psOP.tile(
                        [128, TB], f32, tag="mm", name=f"pj{_pj_n[0]}")
                    _pj_n[0] += 1
                ps = ps_hold[0]
                for d in range(d0, d1):
                    nc.tensor.matmul(
                        ps[:],
                        lhsT=w_tiles[d][:, jt * 128:(jt + 1) * 128],
                        rhs=xt[:, d * TB:(d + 1) * TB],
                        start=(d == 0),
                        stop=(d == NDT - 1),
                    )
                if d1 == NDT:
                    nc.vector.tensor_scalar_add(
                        dstT[:, jt * S + tb * TB: jt * S + (tb + 1) * TB],
                        ps[:],
                        b_tile[:, jt:jt + 1],
                    )

            # ---- v projection: emits [token, feature] tiles directly ----
            # out partitions = 128 tokens of tile tt, free = 256 features
            # (4 heads); evac is one strided copy into vp's 65-wide segments
            # (ones column at offset 64 pre-filled above). b_v folded into
            # b_o on the host.
            def proj_v_chunk(xt, tt):
                c = tt % 4
                vps = psOP.tile([128, JC], f32, tag="mm")
                for d in range(NDT):
                    nc.tensor.matmul(
                        vps[:],
                        lhsT=xt[:, d * TB + c * 128: d * TB + (c + 1) * 128],
                        rhs=wv_sb[d][:, :],
                        start=(d == 0),
                        stop=(d == NDT - 1),
                    )
                dst = vp[:, tt * 2 * VROW:(tt + 1) * 2 * VROW].rearrange(
                    "p (seg c) -> p seg c", seg=4, c=DK + 1)[:, :, 0:DK]
                nc.vector.tensor_copy(
                    dst, vps[:].rearrange("p (seg c) -> p seg c", seg=4, c=DK))

            # ---- attention conveyor ----
            U = {}

            def turn_scores(sb, jt, tt):
                sc = psSC.tile([128, 2 * TB], f32, tag="sc")
                for h, p0 in ((0, 0), (1, 64)):
                    nc.tensor.matmul(
                        sc[:, h * TB:(h + 1) * TB],
                        lhsT=kpT[p0:p0 + DK, jt * S + tt * 128: jt * S + (tt + 1) * 128],
                        rhs=qpT[p0:p0 + DK, jt * S + sb * TB: jt * S + (sb + 1) * TB],
                    )
                ex = exppool.tile([128, 2 * TB], bf16, tag="exp")
                nc.scalar.activation(ex[:], sc[:], AF.Exp, scale=float(1.0 / np.sqrt(DK)))
                return ex

            # mid-kernel norm: the denominator row must reach partition 0 via
            # DMA (no engine can move data across partitions; DMA can't read
            # PSUM, hence the usb staging copy which also releases psU).
            def norm_pair(sb, jt, uA, uB):
                for h, u in ((0, uA), (1, uB)):
                    usb = usbpool.tile([DK + 1, TB], f32, tag="usb")
                    nc.vector.tensor_copy(usb[:], u[:])
                    rc = nrmpool.tile([1, TB], f32, tag="rc")
                    nc.sync.dma_start(rc[:], usb[DK:DK + 1, :])
                    rc2 = nrmpool.tile([1, TB], f32, tag="rc2")
                    nc.vector.reciprocal_approx_fast(rc2[:], rc[:])
                    rb = nrmpool.tile([DK, TB], f32, tag="rb")
                    nc.gpsimd.partition_broadcast(rb[:], rc2[:])
                    if h == 0:
                        nc.vector.tensor_mul(
                            hoT[0:DK, jt * S + sb * TB: jt * S + (sb + 1) * TB],
                            usb[0:DK, :],
                            rb[:],
                        )
                    else:
                        tmp = nrmpool.tile([DK, TB], bf16, tag="tmp")
                        nc.vector.tensor_mul(tmp[:], usb[0:DK, :], rb[:])
                        nc.sync.dma_start(
                            hoT[DK:2 * DK, jt * S + sb * TB: jt * S + (sb + 1) * TB],
                            tmp[:],
                        )

            # endgame norm for pair (3,1), split in two parts so the PE can
            # run filler between them. head3 lands in a base-0 tmp3 tile (the
            # final out-proj contracts it with two K=64 matmuls), so there is
            # no partition-offset hoT DMA on the tail; the partition
            # broadcast of 1/denom runs on the (idle) PE as a K=1 outer
            # product instead of GpSimd.
            _nf = {}

            def norm_fast_p1(uA, uB):
                for h, u in ((0, uA), (1, uB)):
                    usb = usbpool.tile([DK + 1, TB], f32, tag="usb")
                    nc.vector.tensor_copy(usb[:], u[:])
                    rc = nrmpool.tile([1, TB], f32, tag="rc")
                    nc.sync.dma_start(rc[:], usb[DK:DK + 1, :])
                    _nf[h] = (usb, rc)

            def norm_fast_p2():
                rc2s = []
                for h in range(2):
                    rc2 = nrmpool.tile([1, TB], f32, tag="rc2")
                    nc.vector.reciprocal_approx_fast(rc2[:], _nf[h][1][:])
                    rc2b = nrmpool.tile([1, TB], bf16, tag="rc2b")
                    nc.scalar.copy(rc2b[:], rc2[:])
                    rc2s.append(rc2b)
                for h in range(2):
                    usb, _ = _nf[h]
                    rb = psU.tile([DK, TB], f32, tag="U", name=f"rbf{h}")
                    nc.tensor.matmul(rb[:], lhsT=ones64[:], rhs=rc2s[h][:])
                    if h == 0:
                        nc.vector.tensor_mul(
                            hoT[0:DK, S + (NTB - 1) * TB: S + NTB * TB],
                            usb[0:DK, :], rb[:])
                    else:
                        nc.vector.tensor_mul(tmp3[:], usb[0:DK, :], rb[:])

            pend = []

            def pop_pv():
                sb, jt, tt, ex = pend.pop(0)
                if tt == 0:
                    uA = psU.tile([DK + 1, TB], f32, tag="U", name=f"uA_{sb}_{jt}")
                    uB = psU.tile([DK + 1, TB], f32, tag="U", name=f"uB_{sb}_{jt}")
                    U[(sb, jt)] = (uA, uB)
                uA, uB = U[(sb, jt)]
                for h, u in ((0, uA), (1, uB)):
                    o = tt * 2 * VROW + jt * VROW + h * (DK + 1)
                    nc.tensor.matmul(
                        u[:],
                        lhsT=vp[:, o: o + DK + 1],
                        rhs=ex[:, h * TB:(h + 1) * TB],
                        start=(tt == 0),
                        stop=(tt == NTT - 1),
                    )
                if tt == NTT - 1:
                    if (sb, jt) == (NTB - 1, 1):
                        norm_fast_p1(uA, uB)
                    else:
                        norm_pair(sb, jt, uA, uB)

            def push_turn(sb, jt, tt):
                pend.append((sb, jt, tt, turn_scores(sb, jt, tt)))

            # out-proj for query block sb: 8 single-ft groups on psOP
            def outproj_ft(sb, ft, ot):
                op = psOP.tile([128, TB], f32, tag="mm")
                for jt in range(2):
                    nc.tensor.matmul(
                        op[:],
                        lhsT=wo_sb[jt][:, ft * 128:(ft + 1) * 128],
                        rhs=hoT[:, jt * S + sb * TB: jt * S + (sb + 1) * TB],
                        start=(jt == 0),
                        stop=(jt == 1),
                    )
                nc.vector.tensor_scalar_add(
                    ot[:, ft * TB:(ft + 1) * TB], op[:], bo_sb[:, ft:ft + 1]
                )
                if ft == 3 or ft == 7:
                    h0 = 0 if ft == 3 else 512
                    nc.sync.dma_start(
                        out[h0:h0 + 512, sb * TB:(sb + 1) * TB].rearrange(
                            "(ft p) t -> p ft t", p=128),
                        ot[:, (ft - 3) * TB:(ft + 1) * TB].rearrange(
                            "p (ft t) -> p ft t", ft=4),
                    )

            # last block's out-proj: pass 1 (jt0 half, during pair (3,1));
            # pass 2 contracts heads 2/3 with two K=64 matmuls so head3 can
            # stay in a base-0 tile (no partition-offset DMA on the tail).
            def outproj_p1(sb, ft, ot1):
                op = psOP.tile([128, TB], f32, tag="mm")
                nc.tensor.matmul(
                    op[:],
                    lhsT=wo_sb[0][:, ft * 128:(ft + 1) * 128],
                    rhs=hoT[:, sb * TB:(sb + 1) * TB],
                )
                nc.vector.tensor_scalar_add(
                    ot1[:, ft * TB:(ft + 1) * TB], op[:], bo_sb[:, ft:ft + 1]
                )

            def outproj_p2(sb, ft, ot1, ot):
                op = psOP.tile([128, TB], f32, tag="mm")
                nc.tensor.matmul(
                    op[:],
                    lhsT=wo_sb[1][0:64, ft * 128:(ft + 1) * 128],
                    rhs=hoT[0:64, S + sb * TB: S + (sb + 1) * TB],
                    start=True, stop=False,
                )
                nc.tensor.matmul(
                    op[:],
                    lhsT=wo3_sb[:, ft * 128:(ft + 1) * 128],
                    rhs=tmp3[:, :],
                    start=False, stop=True,
                )
                nc.vector.tensor_add(
                    ot[:, ft * TB:(ft + 1) * TB], op[:],
                    ot1[:, ft * TB:(ft + 1) * TB],
                )
                nc.sync.dma_start(
                    out[ft * 128:(ft + 1) * 128, sb * TB:(sb + 1) * TB],
                    ot[:, ft * TB:(ft + 1) * TB],
                )

            # ================= emission =================
            # startup: tb0 with per-jt ordering so pair (0,0) starts ASAP
            xk = load_x(kT, 0, split=4)
            wq_sb = load_w(wq, "wq", NDT, split=2)
            xq = load_x(qT, 0, split=2)
            wv_sb = load_w(wv, "wv", NDT, split=2)
            xv = load_x(vT, 0, split=2)
            warmup(W1)
            proj_jt(xk, wk_sb, bk_sb, kpT, 0, 0)
            warmup(W1B)
            proj_jt(xq, wq_sb, bq_sb, qpT, 0, 0)
            warmup(W1C)
            proj_v_chunk(xv, 0)
            proj_v_chunk(xv, 1)
            push_turn(0, 0, 0)
            push_turn(0, 0, 1)
            proj_v_chunk(xv, 2)
            proj_v_chunk(xv, 3)
            push_turn(0, 0, 2)
            push_turn(0, 0, 3)
            pop_pv()
            pop_pv()
            proj_jt(xk, wk_sb, bk_sb, kpT, 0, 1)
            proj_jt(xq, wq_sb, bq_sb, qpT, 0, 1)
            warmup(W2)
            for tb in range(1, NTB):
                xk = load_x(kT, tb)
                xv = load_x(vT, tb)
                proj_jt(xk, wk_sb, bk_sb, kpT, tb, 0)
                proj_v_chunk(xv, 4 * tb)
                proj_v_chunk(xv, 4 * tb + 1)
                push_turn(0, 0, 4 * tb)
                push_turn(0, 0, 4 * tb + 1)
                pop_pv()
                pop_pv()
                proj_jt(xk, wk_sb, bk_sb, kpT, tb, 1)
                proj_v_chunk(xv, 4 * tb + 2)
                proj_v_chunk(xv, 4 * tb + 3)
                push_turn(0, 0, 4 * tb + 2)
                push_turn(0, 0, 4 * tb + 3)
                pop_pv()
                pop_pv()
            wo_sb = load_w(wo, "wo", 2)
            wo3_sb = const.tile([DK, D], bf16, tag="wo3")
            nc.sync.dma_start(wo3_sb[:], wo[3 * DK:4 * DK, :])

            # steady conveyor over the remaining 7 pairs, two turns per
            # group: [SC SC][weave][PV PV PV PV]. The full-width weave
            # matmul after the row-tiled scores pair absorbs the PE
            # weight-load transition. Weave per group g (0..7):
            #   (s, 1) pairs: q-block s+1 projection, quarter per group 2-5
            #   (s, 0) pairs: out-proj ft g of block s-1
            #   (3, 1): out-proj pass 1 of block 3, 2 fts per group 4-7
            ot1 = const.tile([128, 8 * TB], f32, tag="ot1")
            for sb, jt in [(0, 1)] + [(s, j) for s in range(1, NTB) for j in range(2)]:
                do_op = (jt == 0 and sb > 0)
                do_q = (jt == 1 and sb < NTB - 1)
                do_p1 = (sb, jt) == (NTB - 1, 1)
                if do_op:
                    ot = osbpool.tile([128, 8 * TB], bf16, tag="ot")
                if do_q:
                    xqs = load_x(qT, sb + 1, split=2)
                for g in range(NTT // 2):
                    push_turn(sb, jt, 2 * g)
                    push_turn(sb, jt, 2 * g + 1)
                    # pops FIRST: group 0's pops emit the previous pair's
                    # norm, which the ft-0 out-proj weave reads (deps are
                    # tracked in emission order)
                    pop_pv()
                    pop_pv()
                    if do_p1 and g == NTT // 2 - 1:
                        # drain the last pair's final turns now: their PVs
                        # fit in the exp shadow and the norm staging starts
                        # two turns earlier
                        pop_pv()
                        pop_pv()
                    if do_q and g in (2, 3, 4, 5):
                        hjt, half = divmod(g - 2, 2)
                        proj_jt(xqs, wq_sb, bq_sb, qpT, sb + 1, hjt,
                                d0=half * 4, d1=half * 4 + 4)
                    if do_op:
                        outproj_ft(sb - 1, g, ot)
                    if do_p1 and g >= 4:
                        outproj_p1(NTB - 1, 2 * (g - 4), ot1)
                        outproj_p1(NTB - 1, 2 * (g - 4) + 1, ot1)
            while pend:
                pop_pv()
            # hold the PE clock up across the norm staging latency
            warmup(WDRAIN)
            norm_fast_p2()
            # final out-proj pass 2: split-K matmuls + add-evacs, per-ft DMA
            ot = osbpool.tile([128, 8 * TB], bf16, tag="ot")
            for ft in range(8):
                outproj_p2(NTB - 1, ft, ot1, ot)

    nc.compile()
    return nc


def _get_nc():
    global _NC
    if _NC is None:
        _NC = _build()
    return _NC


def make_in_maps(q, k, v, w_q, b_q, w_k, b_k, w_v, b_v, w_o, b_o):
    import ml_dtypes
    cdt = ml_dtypes.bfloat16
    q = np.asarray(q, np.float32)
    k = np.asarray(k, np.float32)
    v = np.asarray(v, np.float32)
    w_q = np.asarray(w_q, np.float32)
    w_k = np.asarray(w_k, np.float32)
    w_v = np.asarray(w_v, np.float32)
    w_o = np.asarray(w_o, np.float32)
    b_q = np.asarray(b_q, np.float32)
    b_k = np.asarray(b_k, np.float32)
    b_v = np.asarray(b_v, np.float32)
    b_o = np.asarray(b_o, np.float32)
    # v bias folded through attention (rows of P sum to 1 after norm) and
    # the out projection: exact for any inputs.
    b_o_eff = b_o + w_o.T @ b_v

    in_maps = []
    for c in range(NCORES):
        b, g = divmod(c, GROUPS)
        js = slice(g * JC, (g + 1) * JC)
        bias2 = lambda x: x[js].reshape(2, 128).T
        bo2 = (b_o_eff.reshape(8, 128).T if g == 0
               else np.zeros((128, 8), np.float32))
        bias_all = np.concatenate(
            [bias2(b_q), bias2(b_k), np.zeros((128, 2), np.float32), bo2], axis=1)
        in_maps.append({
            "qT": np.ascontiguousarray(q[b].T).astype(cdt),
            "kT": np.ascontiguousarray(k[b].T).astype(cdt),
            "vT": np.ascontiguousarray(v[b].T).astype(cdt),
            "wq": np.ascontiguousarray(w_q[:, js]).astype(cdt),
            "wk": np.ascontiguousarray(w_k[:, js]).astype(cdt),
            "wv": np.ascontiguousarray(w_v[:, js]).astype(cdt),
            "wo": np.ascontiguousarray(w_o[js, :]).astype(cdt),
            "bias": np.ascontiguousarray(bias_all, dtype=np.float32),
        })
    return in_maps


def gather(results):
    out = np.zeros((B, S, D), np.float32)
    for c in range(NCORES):
        b = c // GROUPS
        out[b] += results[c]["out"].T.astype(np.float32)
    return out


def kernel(q, k, v, w_q, b_q, w_k, b_k, w_v, b_v, w_o, b_o, _trace=False):
    from concourse.bass_utils import run_bass_kernel_spmd

    nc = _get_nc()
    in_maps = make_in_maps(q, k, v, w_q, b_q, w_k, b_k, w_v, b_v, w_o, b_o)
    res = run_bass_kernel_spmd(nc, in_maps, core_ids=list(range(NCORES)), trace=_trace)
    out = gather(res.results)
    if _trace:
        kernel.last_exec_time_ns = res.exec_time_ns
        kernel.last_results = res
    return out
